# revision 3
# baseline (speedup 1.0000x reference)
"""Bass/Trainium2 kernel for nn_BiMultiHeadAttention (sparse_attention).

Sharding: 8 cores = 4 batches x 2 head-pairs. Each core handles one batch and
2 of the 4 heads end-to-end; host sums the two half-partials per batch
(tensor-parallel output reduction) and adds biases.

Device math per core (B=one batch, heads h0,h1; T=64 txt, N=21760 img, V=256):
  awT[ht, i] = (v @ Bm)[i, ht] * c_lvl + caw     (Bm = SCALE * vp_h^T k_h^T, host)
  pyramid refinement done in-place on awT (conv via PE block-diag matmul,
  bilinear up via DVE lerps; the (1 - n*alpha/2) "cur" scaling is pre-folded
  into the phase-A evacuation scale c_lvl)
  E = exp(awT)  (one ACT pass; accum_out gives p_l row sums)
  out_v partial = (E/colsum_head) @ Wvl          (Wvl = val_l_h ovp_h^T, host)
  out_l partial = diag(1/rowsum) E v @ Mm        (Mm = vvp_h^T olp_h^T, host)
"""
import numpy as np
import ml_dtypes

import concourse.bass as bass
import concourse.mybir as mybir
from concourse.tile import TileContext

F32 = mybir.dt.float32
BF16 = mybir.dt.bfloat16

NUM_HEADS = 4
HEAD_DIM = 256
V_DIM = 256
L_DIM = 768
N_TXT = 64
SCALE = HEAD_DIM ** -0.5
SPATIAL = [(128, 128), (64, 64), (32, 32), (16, 16)]
N_IMG = sum(h * w for h, w in SPATIAL)          # 21760
LV_OFF = [0, 16384, 20480, 21504]
NT = N_IMG // 128                                # 170 img tiles
P = 128

# coef column indices
C_UA, C_U3A, C_U4A, C_UB, C_U3B, C_U4B, C_AH, C_C1, C_C2 = range(9)
NCOEF = 9

# conv chunking: (level producing lo_i, out rows per chunk)
CONV_PLAN = [  # (i, H_out, W_out, rows_per_chunk)
    (1, 64, 64, 8),
    (2, 32, 32, 16),
    (3, 16, 16, 16),
]
LO_OFF = {1: 0, 2: 4096, 3: 5120}
LO_LEN = 5376


def _split_excess_waits(nc, keep_on_inst=1):
    """This container's walrus rejects instructions with >1 sem wait
    (per-opcode limits; Drain/NoOp accept only 1). Move excess waits onto
    InstNoOp's inserted just before, on the same engine; a wait executed
    earlier on the same engine preserves semantics."""
    n = 0
    for fn in nc.m.functions:
        for blk in fn.blocks:
            out = []
            for inst in blk.instructions:
                si = getattr(inst, "sync_info", None)
                waits = list(si.on_wait) if (si is not None and si.on_wait) else []
                if len(waits) > keep_on_inst:
                    excess, keep = waits[:-keep_on_inst], waits[-keep_on_inst:]
                    for k, w in enumerate(excess):
                        nop = mybir.InstNoOp(
                            name=f"{inst.name}-wsplit-{k}", ins=[], outs=[])
                        nop.engine = inst.engine
                        nop.sync_info = mybir.SyncInfo(on_wait=[w], on_update=[])
                        out.append(nop)
                        n += 1
                    inst.sync_info = mybir.SyncInfo(
                        on_wait=keep, on_update=list(si.on_update or []))
                out.append(inst)
            blk.instructions = out
    return n


def build_nc():
    nc = bass.Bass()
    AF = mybir.ActivationFunctionType
    ALU = mybir.AluOpType

    vbf_d = nc.dram_tensor("vbf", [N_IMG, V_DIM], BF16, kind="ExternalInput")
    Bm_d = nc.dram_tensor("Bm", [P, 2, P], BF16, kind="ExternalInput")
    cawc_d = nc.dram_tensor("cawc", [P, 2], F32, kind="ExternalInput")
    Wvl_d = nc.dram_tensor("Wvl", [P, 256], BF16, kind="ExternalInput")
    Mm_d = nc.dram_tensor("Mm", [P, 2, 2, L_DIM], F32, kind="ExternalInput")
    cw_d = nc.dram_tensor("cw", [P, 2, 9, P], BF16, kind="ExternalInput")
    cbias_d = nc.dram_tensor("cbias", [P, 1], F32, kind="ExternalInput")
    coef_d = nc.dram_tensor("coef", [P, NCOEF], F32, kind="ExternalInput")
    bones_d = nc.dram_tensor("bones", [P, P], BF16, kind="ExternalInput")
    identf_d = nc.dram_tensor("identf", [P, P], F32, kind="ExternalInput")
    ov_d = nc.dram_tensor("ov", [N_IMG, 256], F32, kind="ExternalOutput")
    ol_d = nc.dram_tensor("ol", [N_TXT, L_DIM], F32, kind="ExternalOutput")

    with TileContext(nc) as tc:
        with tc.tile_pool(name="const", bufs=1) as cp, \
             tc.tile_pool(name="big", bufs=1) as bigp:
            # constants
            Bm = cp.tile([P, 2, P], BF16, tag="Bm")
            cawc = cp.tile([P, 2], F32, tag="cawc")
            Wvl = cp.tile([P, 256], BF16, tag="Wvl")
            Mm = cp.tile([P, 2, 2, L_DIM], F32, tag="Mm")
            cw = cp.tile([P, 2, 9, P], BF16, tag="cw")
            cbias = cp.tile([P, 1], F32, tag="cbias")
            coef = cp.tile([P, NCOEF], F32, tag="coef")
            bones = cp.tile([P, P], BF16, tag="bones")
            identf = cp.tile([P, P], F32, tag="identf")
            for t, d in [(Bm, Bm_d), (cawc, cawc_d), (Wvl, Wvl_d), (Mm, Mm_d),
                         (cw, cw_d), (cbias, cbias_d), (coef, coef_d),
                         (bones, bones_d), (identf, identf_d)]:
                nc.sync.dma_start(out=t[:], in_=d[:])

            def co(j):  # coef column AP [P, 1]
                return coef[:, j:j + 1]

            # persistent big buffers
            awT = bigp.tile([P, N_IMG], F32, tag="awT")
            lo_s = bigp.tile([P, LO_LEN], F32, tag="lo_s")
            spart = bigp.tile([P, NT], F32, tag="spart")

            def lvl_of_tile(t):
                return 0 if t < 128 else (1 if t < 160 else (2 if t < 168 else 3))

            # ---------------- Phase A: awT = scaled (v @ Bm)^T + bias -------
            with tc.tile_pool(name="pa_sb", bufs=3) as pa, \
                 tc.tile_pool(name="pa_ps", bufs=3, space="PSUM") as pap:
                for t in range(NT):
                    r0 = t * 128
                    vT = pa.tile([P, 2, P], BF16, tag="vT")
                    for c in range(2):
                        nc.sync.dma_start(
                            out=vT[:, c, :],
                            in_=vbf_d[r0:r0 + 128, c * 128:(c + 1) * 128],
                            transpose=True)
                    awp = pap.tile([P, P], F32, tag="awp")
                    for c in range(2):
                        nc.tensor.matmul(awp[:], Bm[:, c, :], vT[:, c, :],
                                         start=(c == 0), stop=(c == 1))
                    lv = lvl_of_tile(t)
                    ccol = C_C1 if lv in (0, 3) else C_C2
                    bcol = 0 if lv in (0, 3) else 1
                    nc.scalar.activation(
                        awT[:, r0:r0 + 128], awp[:], AF.Identity,
                        bias=cawc[:, bcol:bcol + 1], scale=co(ccol))

            # ---------------- Phase B: pyramid refinement in-place ----------
            with tc.tile_pool(name="pb_sb", bufs=1) as pb, \
                 tc.tile_pool(name="pb_ps", bufs=2, space="PSUM") as pbp:
                # 1) convs lo_i = ah * (conv_s2(L[i-1]_orig) + down_b)
                for (i, HO, WO, RC) in CONV_PLAN:
                    HI, WI = 2 * HO, 2 * WO
                    src_lv = i - 1
                    var = 0 if src_lv in (0, 3) else 1  # cw variant undoes c-scale
                    pbuf = pb.tile([P, HI + 2, WI + 2], BF16, tag="scratch")
                    pb3 = pbuf[:]
                    nc.vector.memset(pb3[:, 0, :], 0)
                    nc.vector.memset(pb3[:, HI + 1, :], 0)
                    nc.vector.memset(pb3[:, 1:HI + 1, 0:1], 0)
                    nc.vector.memset(pb3[:, 1:HI + 1, WI + 1:WI + 2], 0)
                    src = awT[:, LV_OFF[src_lv]:LV_OFF[src_lv] + HI * WI] \
                        .rearrange("p (h w) -> p h w", h=HI)
                    nc.vector.tensor_copy(pb3[:, 1:HI + 1, 1:WI + 1], src)
                    lo3 = lo_s[:, LO_OFF[i]:LO_OFF[i] + HO * WO] \
                        .rearrange("p (h w) -> p h w", h=HO)
                    for y0 in range(0, HO, RC):
                        cps = pbp.tile([P, RC * WO], F32, tag="cps")
                        cps3 = cps[:].rearrange("p (h w) -> p h w", h=RC)
                        for kk in range(9):
                            ky, kx = kk // 3, kk % 3
                            rhs = pb3[:, ky + 2 * y0: ky + 2 * y0 + 2 * RC: 2,
                                      kx: kx + 2 * WO: 2]
                            nc.tensor.matmul(cps3[:], cw[:, var, kk, :], rhs,
                                             start=(kk == 0), stop=(kk == 8))
                        nc.scalar.activation(
                            lo3[:, y0:y0 + RC, :], cps3[:], AF.Identity,
                            bias=cbias[:, 0:1], scale=co(C_AH))

                # 2) per-level in-place combine, order 0..3
                for i in range(4):
                    H, W = SPATIAL[i]
                    V = awT[:, LV_OFF[i]:LV_OFF[i] + H * W] \
                        .rearrange("p (h w) -> p h w", h=H)
                    if i < 3:
                        h, w = H // 2, W // 2
                        S = awT[:, LV_OFF[i + 1]:LV_OFF[i + 1] + h * w] \
                            .rearrange("p (h w) -> p h w", h=h)
                        cu, cu3, cu4 = ((C_UA, C_U3A, C_U4A) if i < 2
                                        else (C_UB, C_U3B, C_U4B))
                        rp4 = pb.tile([P, H, w], BF16, tag="rp4")
                        r4 = rp4[:]
                        # row pass (x4 scaled): even rows 2j <- 3*s[j]+s[j-1]
                        nc.vector.scalar_tensor_tensor(
                            r4[:, 2:H:2, :], S[:, 1:h, :], 3.0, S[:, 0:h - 1, :],
                            ALU.mult, ALU.add)
                        nc.vector.scalar_tensor_tensor(
                            r4[:, 1:H - 1:2, :], S[:, 0:h - 1, :], 3.0, S[:, 1:h, :],
                            ALU.mult, ALU.add)
                        nc.vector.tensor_scalar_mul(r4[:, 0, :], S[:, 0, :], 4.0)
                        nc.vector.tensor_scalar_mul(r4[:, H - 1, :], S[:, h - 1, :], 4.0)
                        # col pass fused with accumulate into V
                        nc.vector.scalar_tensor_tensor(
                            V[:, :, 0:1], r4[:, :, 0:1], co(cu4), V[:, :, 0:1],
                            ALU.mult, ALU.add)
                        nc.vector.scalar_tensor_tensor(
                            V[:, :, W - 1:W], r4[:, :, w - 1:w], co(cu4),
                            V[:, :, W - 1:W], ALU.mult, ALU.add)
                        Ve = V[:, :, 2:W:2]
                        nc.vector.scalar_tensor_tensor(
                            Ve, r4[:, :, 1:w], co(cu3), Ve, ALU.mult, ALU.add)
                        nc.vector.scalar_tensor_tensor(
                            Ve, r4[:, :, 0:w - 1], co(cu), Ve, ALU.mult, ALU.add)
                        Vo = V[:, :, 1:W - 1:2]
                        nc.vector.scalar_tensor_tensor(
                            Vo, r4[:, :, 0:w - 1], co(cu3), Vo, ALU.mult, ALU.add)
                        nc.vector.scalar_tensor_tensor(
                            Vo, r4[:, :, 1:w], co(cu), Vo, ALU.mult, ALU.add)
                    if i > 0:
                        lo = lo_s[:, LO_OFF[i]:LO_OFF[i] + H * W] \
                            .rearrange("p (h w) -> p h w", h=H)
                        nc.vector.scalar_tensor_tensor(
                            V[:], lo, 0.0, V[:], ALU.bypass, ALU.add)

            # ---------------- Phase D: softmaxes + output matmuls -----------
            with tc.tile_pool(name="pd_sb", bufs=3) as pd, \
                 tc.tile_pool(name="pg_ps", bufs=1, space="PSUM") as pgp:
              Gps = pgp.tile([P, 256], F32, tag="Gps")
              with tc.tile_pool(name="pd_ps", bufs=2, space="PSUM") as pdp:
                for t in range(NT):
                    r0 = t * 128
                    vn = pd.tile([P, V_DIM], BF16, tag="vn")
                    nc.sync.dma_start(out=vn[:], in_=vbf_d[r0:r0 + 128, :])
                    Et = pd.tile([P, P], BF16, tag="Et")
                    nc.scalar.activation(
                        Et[:], awT[:, r0:r0 + 128], AF.Exp,
                        accum_out=spart[:, t:t + 1])
                    # p_v: per-head column sums (broadcast via block-ones)
                    Sps = pdp.tile([P, P], F32, tag="Sps")
                    nc.tensor.matmul(Sps[:], bones[:], Et[:], start=True, stop=True)
                    Rs = pd.tile([P, P], F32, tag="Rs")
                    nc.vector.reciprocal(Rs[:], Sps[:])
                    Pv = pd.tile([P, P], BF16, tag="Pv")
                    nc.vector.scalar_tensor_tensor(
                        Pv[:], Et[:], 0.0, Rs[:], ALU.bypass, ALU.mult)
                    ovps = pdp.tile([P, 256], F32, tag="ovps")
                    nc.tensor.matmul(ovps[:], Pv[:], Wvl[:], start=True, stop=True)
                    ovs = pd.tile([P, 256], F32, tag="ovs")
                    nc.scalar.copy(ovs[:], ovps[:])
                    nc.sync.dma_start(out=ov_d[r0:r0 + 128, :], in_=ovs[:])
                    # G accumulation (p_l un-normalized): ET = Et^T via xbar
                    ETs = pd.tile([P, P], BF16, tag="ETs")
                    nc.sync.dma_start(out=ETs[:], in_=Et[:], transpose=True)
                    nc.tensor.matmul(Gps[:], ETs[:], vn[:],
                                     start=(t == 0), stop=(t == NT - 1),
                                     skip_group_check=True)

              # tail: normalize G rows, project to out_l
              with tc.tile_pool(name="pt_ps", bufs=2, space="PSUM") as pdp:
                sl = pd.tile([P, 1], F32, tag="sl")
                nc.vector.tensor_reduce(sl[:], spart[:], mybir.AxisListType.X,
                                        ALU.add)
                rsl = pd.tile([P, 1], F32, tag="rsl")
                nc.vector.reciprocal(rsl[:], sl[:])
                Gsb = pd.tile([P, 256], F32, tag="Gsb")
                nc.scalar.activation(Gsb[:], Gps[:], AF.Identity,
                                     scale=rsl[:, 0:1])
                GT = pd.tile([P, 2, P], F32, tag="GT")
                for c in range(2):
                    gtp = pdp.tile([P, P], F32, tag="gtp")
                    nc.tensor.transpose(gtp[:], Gsb[:, c * 128:(c + 1) * 128],
                                        identf[:])
                    nc.vector.tensor_copy(GT[:, c, :], gtp[:])
                ols = pd.tile([N_TXT, L_DIM], F32, tag="ols")
                for n in range(2):
                    olps = pdp.tile([N_TXT, 384], F32, tag="olps")
                    for h in range(2):
                        for c in range(2):
                            nc.tensor.matmul(
                                olps[:], GT[:, c, h * 64:(h + 1) * 64],
                                Mm[:, h, c, n * 384:(n + 1) * 384],
                                start=(h == 0 and c == 0),
                                stop=(h == 1 and c == 1),
                                skip_group_check=True)
                    nc.scalar.copy(ols[:, n * 384:(n + 1) * 384], olps[:])
                nc.sync.dma_start(out=ol_d[:], in_=ols[:])

    _split_excess_waits(nc)
    return nc


_NC_CACHE = None


def _get_nc():
    global _NC_CACHE
    if _NC_CACHE is None:
        _NC_CACHE = build_nc()
    return _NC_CACHE


def _host_prep(inputs):
    """Build per-core input maps."""
    f = np.float32
    v = np.asarray(inputs['v'], f)
    l = np.asarray(inputs['l'], f)
    alpha = float(np.asarray(inputs['sim_parm']).reshape(-1)[0])
    vp_w = np.asarray(inputs['vp_w'], f); vp_b = np.asarray(inputs['vp_b'], f)
    lp_w = np.asarray(inputs['lp_w'], f); lp_b = np.asarray(inputs['lp_b'], f)
    vvp_w = np.asarray(inputs['vvp_w'], f)
    vlp_w = np.asarray(inputs['vlp_w'], f); vlp_b = np.asarray(inputs['vlp_b'], f)
    ovp_w = np.asarray(inputs['ovp_w'], f)
    olp_w = np.asarray(inputs['olp_w'], f)
    down_w = np.asarray(inputs['down_w'], f)
    down_b = np.asarray(inputs['down_b'], f)

    ah = 0.5 * alpha
    c1, c2 = 1.0 - ah, 1.0 - 2.0 * ah
    ua, ub = ah / 16.0 / c2, ah / 16.0 / c1
    coef = np.zeros((P, NCOEF), f)
    coef[:, [C_UA, C_U3A, C_U4A, C_UB, C_U3B, C_U4B, C_AH, C_C1, C_C2]] = \
        [ua, 3 * ua, 4 * ua, ub, 3 * ub, 4 * ub, ah, c1, c2]

    cw = np.zeros((P, 2, 9, P), f)
    for kk in range(9):
        ky, kx = kk // 3, kk % 3
        wt = down_w[:, :, ky, kx].T                   # [in, out]
        for h in range(2):
            cw[h * 64:(h + 1) * 64, 0, kk, h * 64:(h + 1) * 64] = wt / c1
            cw[h * 64:(h + 1) * 64, 1, kk, h * 64:(h + 1) * 64] = wt / c2
    cbias = np.tile(ah * down_b, 2).reshape(P, 1).astype(f)

    bones = np.zeros((P, P), f)
    bones[:64, :64] = 1.0
    bones[64:, 64:] = 1.0
    identf = np.eye(P, dtype=f)

    # per-head (batch-independent) matrices
    Mh = []   # [V_DIM, L_DIM]
    for h in range(NUM_HEADS):
        sl_h = slice(h * HEAD_DIM, (h + 1) * HEAD_DIM)
        Mh.append(vvp_w[sl_h].T @ olp_w[:, sl_h].T)

    bf = ml_dtypes.bfloat16
    shared = {
        'cw': cw.astype(bf), 'cbias': cbias, 'coef': coef,
        'bones': bones.astype(bf), 'identf': identf,
    }
    in_maps = []
    for core in range(8):
        b, hp = core // 2, core % 2
        heads = [2 * hp, 2 * hp + 1]
        l_b = l[b]
        B_cat = np.empty((V_DIM, P), f)
        caw = np.empty((P,), f)
        Wvl_cat = np.empty((P, 256), f)
        Mm = np.empty((P, 2, 2, L_DIM), f)
        for j, h in enumerate(heads):
            sl_h = slice(h * HEAD_DIM, (h + 1) * HEAD_DIM)
            k_h = l_b @ lp_w[sl_h].T + lp_b[sl_h]                # [64, 256]
            val_l = l_b @ vlp_w[sl_h].T + np.asarray(inputs['vlp_b'], f)[sl_h]
            B_cat[:, j * 64:(j + 1) * 64] = SCALE * (vp_w[sl_h].T @ k_h.T)
            caw[j * 64:(j + 1) * 64] = SCALE * (vp_b[sl_h] @ k_h.T)
            Wvl_cat[j * 64:(j + 1) * 64] = val_l @ ovp_w[:, sl_h].T
            M = Mh[h]                                            # [256, 768]
            Mm[:, j, 0] = M[:128]
            Mm[:, j, 1] = M[128:]
        Bm = np.empty((P, 2, P), f)
        Bm[:, 0, :] = B_cat[:128]
        Bm[:, 1, :] = B_cat[128:]
        cawc = np.stack([c1 * caw, c2 * caw], axis=1).astype(f)
        in_maps.append({
            'vbf': np.ascontiguousarray(v[b]).astype(bf),
            'Bm': Bm.astype(bf),
            'cawc': cawc,
            'Wvl': Wvl_cat.astype(bf),
            'Mm': Mm,
            **shared,
        })
    return in_maps


def _unshard(inputs, results):
    f = np.float32
    ovp_b = np.asarray(inputs['ovp_b'], f)
    olp_b = np.asarray(inputs['olp_b'], f)
    vvp_b = np.asarray(inputs['vvp_b'], f)
    olp_w = np.asarray(inputs['olp_w'], f)
    ol_const = olp_b + vvp_b @ olp_w.T
    out_v = np.empty((4, N_IMG, 256), f)
    out_l = np.empty((4, N_TXT, L_DIM), f)
    for b in range(4):
        out_v[b] = results[2 * b]['ov'] + results[2 * b + 1]['ov'] + ovp_b
        out_l[b] = results[2 * b]['ol'] + results[2 * b + 1]['ol'] + ol_const
    return out_v, out_l


def _run(inputs, trace=False):
    from concourse.bass_utils import run_bass_kernel_spmd
    in_maps = _host_prep(inputs)
    nc = _get_nc()
    r = run_bass_kernel_spmd(nc, in_maps, list(range(8)), trace=trace)
    return _unshard(inputs, r.results), r


def kernel(**inputs):
    (out_v, out_l), _ = _run(inputs, trace=False)
    return out_v, out_l
